# revision 61
# baseline (speedup 1.0000x reference)
"""DiT block kernel for 8 Trainium2 NeuronCores.

Sharding: data-parallel over (batch, seq-half) -> 8 shards, no collectives.
Each core gets x[b] rotated so its 512 query tokens are rows 0:511; K/V are
computed over the full (rotated) 1024-token sequence, so attention needs no
cross-core communication.

v4: the adaLN modulation is folded into the weights HOST-side per core
(c @ adaLN_w is a tiny fp32 matmul): scale rows of wqkv/w1, shift becomes a
bias row (extra rank-1 matmul for QKV, an exact fp32 ACT bias for fc1), and
the msa/mlp gates scale the columns of wout/w2. On-device: LayerNorm via
bn_stats, fp8e4 DoubleRow GEMMs (4x fewer PE cycles), bf16 transposes / rope
/ attention. Weights pre-scaled x64 into e4m3's normal range; 1/64 folded
into downstream scales (exp scale, gelu scale, residual-add scale).
"""

import sys

sys.path.insert(0, "/opt/trn_rl_repo")

import ml_dtypes
import numpy as np

import concourse.bass as bass
import concourse.mybir as mybir
from concourse.bass_utils import run_bass_kernel_spmd
from concourse.masks import make_identity
from concourse.tile import TileContext
from concourse.vector_clock import ScopedClock, VectorClock

# ---------------------------------------------------------------------------
# Walrus in this container caps sync-wait commands per CTRL instruction at a
# small number; Tile's stock tail drain collects one wait per live proc and
# trips that cap. Split the final waits across one SP NOP per proc instead.
_orig_drain_and_barrier = TileContext._drain_and_barrier


def _split_drain_and_barrier(self, tick_clock, wait_clock):
    gc_list = list(tick_clock.global_clock)
    for p, tick in enumerate(gc_list):
        if tick > 0:
            partial = [0] * len(gc_list)
            partial[p] = tick
            nop = self.nc.sync.nop()
            wait_clock.add_sem_waits(nop.ins, ScopedClock({None: VectorClock(partial)}))
    drain_inst = self.nc.sync.drain()
    req = ScopedClock({None: tick_clock.global_clock.copy()})
    cur = ScopedClock({None: tick_clock.global_clock.copy()})
    wait_clock.add_sem_waits(drain_inst.ins, req, cur)
    self.nc.all_engine_barrier()
    popped = self.nc._tile_sem_poison_stack.pop()
    assert popped is self._sem_poison
    self.nc.clear_and_free_semaphores(list(self.sems.allocated().values()))
    self.nc.all_engine_barrier()


TileContext._drain_and_barrier = _split_drain_and_barrier

# This walrus also caps waits per *compute/DMA* instruction (the S3_LW struct
# allows a single wait). Intercept every instruction Tile commits to a basic
# block and spill all but the last wait onto preceding same-engine NOPs.
_nop_proto = None


def _get_nop_proto():
    global _nop_proto
    if _nop_proto is None:
        scratch = bass.Bass()
        _nop_proto = scratch.sync.nop().ins
        _nop_proto.sync_info = None
    return _nop_proto


_orig_add_instruction = TileContext._add_instruction


def _add_instruction_capped(self, inst):
    si = inst.sync_info
    if si is not None and si.on_wait is not None and len(si.on_wait) > 1:
        waits = list(si.on_wait)
        si.on_wait = waits[-1:]
        import copy as _copy

        for w in waits[:-1]:
            nop = _copy.deepcopy(_get_nop_proto())
            nop.name = self.nc.get_next_instruction_name()
            nop.engine = inst.engine
            nop.sync_info = mybir.SyncInfo(on_wait=[w], on_update=[])
            _orig_add_instruction(self, nop)
    _orig_add_instruction(self, inst)


TileContext._add_instruction = _add_instruction_capped

# Capture the Tile scheduler's modeled makespan (cost-model ns) per block —
# the only timing signal available in this container (no NTFF profiling).
MODELED_NS = []
from concourse.bass_interp import CoreSim as _CoreSim

_orig_coresim_simulate = _CoreSim.simulate


def _simulate_capture(self, *a, **k):
    r = _orig_coresim_simulate(self, *a, **k)
    try:
        if self.is_scheduling_pass():
            MODELED_NS.append((getattr(self, "name", "?"), int(self.time)))
    except Exception:
        pass
    return r


_CoreSim.simulate = _simulate_capture


def _publish_perfetto_local(self):
    """Dump the scheduling-sim perfetto trace to a local file (no upload)."""
    import os as _os

    if self.perfetto is None:
        return
    path = _os.environ.get("BASS_MODEL_TRACE_PATH", "/tmp/model_trace.pftrace")
    with open(path, "wb") as f:
        f.write(bytes(self.perfetto.take_serialized()))
    print(f"[kernel] modeled trace written to {path}")


_CoreSim.publish_perfetto = _publish_perfetto_local

# ---------------------------------------------------------------------------

FP32 = mybir.dt.float32
BF16 = mybir.dt.bfloat16
FP8 = mybir.dt.float8e4
AF = mybir.ActivationFunctionType
ALU = mybir.AluOpType
AX = mybir.AxisListType
PM = mybir.MatmulPerfMode

D = 1024
H = 16
HD = 64
S = 1024
SQ = 512  # query tokens per core
MLP = 4096
COND = 128
NT = S // 128  # 8 token tiles
NTQ = SQ // 128  # 4 query token tiles
LN_EPS = 1e-5
N_CORES = 8
SW = 64.0  # fp8 weight pre-scale (host side); 1/SW folded into act scales
RSW = 1.0 / SW


def _build_nc(reps=1):
    import os as _os

    trace_sim = bool(_os.environ.get("BASS_MODEL_TRACE"))
    nc = bass.Bass()

    xb = nc.dram_tensor("xb", [S, D], BF16, kind="ExternalInput")
    wqkv = nc.dram_tensor("wqkv", [D, 3 * D], FP8, kind="ExternalInput")
    bqkv = nc.dram_tensor("bqkv", [1, 3 * D], FP8, kind="ExternalInput")
    wout = nc.dram_tensor("wout", [D, D], FP8, kind="ExternalInput")
    w1d = nc.dram_tensor("w1", [D, MLP], FP8, kind="ExternalInput")
    b1g = nc.dram_tensor("b1g", [128, 32], FP32, kind="ExternalInput")
    w2d = nc.dram_tensor("w2", [MLP, D], FP8, kind="ExternalInput")
    cosr = nc.dram_tensor("cosr", [S, 512], BF16, kind="ExternalInput")
    sinr = nc.dram_tensor("sinr", [S, 512], BF16, kind="ExternalInput")
    outd = nc.dram_tensor("out", [SQ, D], FP32, kind="ExternalOutput")

    with TileContext(nc, trace_sim=trace_sim) as tc:
        def _emit_body():
            # ------------------------------------------------------ persistent
            const_cm = tc.tile_pool(name="const", bufs=1)
            const = const_cm.__enter__()
            ident = const.tile([128, 128], BF16, tag="ident")
            make_identity(nc, ident)
            # PE p-state warmup: dummy transposes while DMAs stream, so the
            # 3us ramp completes before the first real matmul.
            warm_cm = tc.tile_pool(name="warmps", bufs=1, space="PSUM")
            warmp = warm_cm.__enter__()
            warm = warmp.tile([128, 8, 128], BF16, tag="warm")
            for blk in range(8):
                nc.tensor.transpose(warm[:, blk, :], ident, ident)
            ones128 = const.tile([128, 128], BF16, tag="ones128")
            nc.vector.memset(ones128, 1.0)
            ones_r = const.tile([1, 128], BF16, tag="ones")
            nc.vector.tensor_copy(ones_r, ones128[0:1, :])
            ones8 = const.tile([1, 128], FP8, tag="ones8")
            nc.vector.memset(ones8, 1.0)
            epst = const.tile([128, 1], FP32, tag="eps")
            nc.vector.memset(epst, LN_EPS)
            bq_sb = const.tile([1, 3 * D], FP8, tag="bq")
            nc.gpsimd.dma_start(out=bq_sb, in_=bqkv[:, :])
            b1g_sb = const.tile([128, 32], FP32, tag="b1g")

            xmT_cm = tc.tile_pool(name="xmTp", bufs=1)
            xmTp = xmT_cm.__enter__()
            xmT = xmTp.tile([128, 8, S], FP8, tag="xmT")

            s1x_cm = tc.tile_pool(name="s1x", bufs=8, side="right")
            s1x = s1x_cm.__enter__()

            # stage-2 weight pools
            s2wk_cm = tc.tile_pool(name="s2wk", bufs=1)
            s2wk = s2wk_cm.__enter__()
            s2wq_cm = tc.tile_pool(name="s2wq", bufs=1)
            s2wq = s2wq_cm.__enter__()

            xts = []
            xt = s1x.tile([128, D], BF16, tag="xt", name="xt")
            nc.sync.dma_start(out=xt[:, 0:512], in_=xb[0:128, 0:512])
            nc.sync.dma_start(out=xt[:, 512:1024], in_=xb[0:128, 512:1024])
            xts.append(xt)
            for tt in range(1, 4):
                xt = s1x.tile([128, D], BF16, tag="xt", name="xt")
                nc.sync.dma_start(out=xt, in_=xb[tt * 128 : (tt + 1) * 128, :])
                xts.append(xt)

            # q-weight stream interleaves with the remaining x tiles
            wAq = s2wq.tile([128, 8, 1024], FP8, tag="wA", name="wAq")
            nc.sync.dma_start(
                out=wAq, in_=wqkv[:, 0:1024].rearrange("(a p) n -> p a n", p=128)
            )
            for tt in range(4, NT):
                xt = s1x.tile([128, D], BF16, tag="xt", name="xt")
                nc.sync.dma_start(out=xt, in_=xb[tt * 128 : (tt + 1) * 128, :])
                xts.append(xt)

            # k/v weights stream next on the sync queue
            wAk = s2wk.tile([128, 8, 1024], FP8, tag="wAk", name="wAk")
            nc.sync.dma_start(
                out=wAk, in_=wqkv[:, D : 2 * D].rearrange("(a p) n -> p a n", p=128)
            )
            wB = s2wk.tile([128, 8, 1024], FP8, tag="wB", name="wB")
            nc.sync.dma_start(
                out=wB,
                in_=wqkv[:, 2 * D : 3 * D].rearrange("(a p) n -> p a n", p=128),
            )

            # ---------------------------------------------- LN + transpose
            def ln_transpose(xt, stats, tpp, xmp, dest, deng):
                """LayerNorm(xt) (modulation folded into weights host-side),
                transposed into dest (fp8) with one plain copy."""
                bst = stats.tile([128, 2, 6], FP32, tag="bst")
                nc.vector.bn_stats(bst[:, 0, :], xt[:, 0:512])
                nc.vector.bn_stats(bst[:, 1, :], xt[:, 512:1024])
                mv = stats.tile([128, 2], FP32, tag="mv")
                nc.vector.bn_aggr(mv, bst.rearrange("p a b -> p (a b)"))
                std = stats.tile([128, 1], FP32, tag="std")
                nc.scalar.activation(
                    out=std, in_=mv[:, 1:2], func=AF.Sqrt, bias=epst
                )
                rstd = stats.tile([128, 1], FP32, tag="rstd")
                nc.vector.reciprocal(rstd, std)
                bmu = stats.tile([128, 1], FP32, tag="bmu")
                nc.vector.tensor_scalar(
                    out=bmu, in0=mv[:, 0:1], scalar1=rstd, scalar2=-1.0,
                    op0=ALU.mult, op1=ALU.mult,
                )
                xm = xmp.tile([128, D], BF16, tag="xm")
                nc.scalar.activation(
                    out=xm, in_=xt, func=AF.Identity, scale=rstd, bias=bmu
                )
                pt = tpp.tile([128, 8, 128], BF16, tag="tp")
                for blk in range(8):
                    nc.tensor.transpose(
                        pt[:, blk, :], xm[:, blk * 128 : (blk + 1) * 128], ident
                    )
                deng(dest, pt)

            def act_copy(dst, src):
                nc.scalar.copy(dst, src)

            def dve_copy(dst, src):
                nc.vector.tensor_copy(dst, src)

            with (
                tc.tile_pool(name="s1st", bufs=4) as s1st,
                tc.tile_pool(name="s1xm", bufs=3) as s1xm,
                tc.tile_pool(name="s1tp", bufs=2, space="PSUM", side="right") as s1tp,
            ):
                for tt in range(NT):
                    ln_transpose(
                        xts[tt], s1st, s1tp, s1xm,
                        xmT[:, :, tt * 128 : (tt + 1) * 128],
                        act_copy if tt % 2 == 0 else dve_copy,
                    )
            s1x_cm.__exit__(None, None, None)
            warm_cm.__exit__(None, None, None)

            # MLP weight pools reserve early (below the attention-phase pools
            # in the right stack); their DMAs are emitted at attention start.
            s6w1_cm = tc.tile_pool(name="s6w1", bufs=1, side="right")
            s6w1 = s6w1_cm.__enter__()
            s6w2_cm = tc.tile_pool(name="s6w2", bufs=1, side="right")
            s6w2 = s6w2_cm.__enter__()

            s2ps_cm = tc.tile_pool(name="s2ps", bufs=2, space="PSUM")
            s2ps = s2ps_cm.__enter__()
            s2tp_cm = tc.tile_pool(name="s2tp", bufs=2, space="PSUM")
            s2tp = s2tp_cm.__enter__()

            s2c_cm = tc.tile_pool(name="s2c", bufs=3, side="right")
            s2c = s2c_cm.__enter__()
            s2r_cm = tc.tile_pool(name="s2r", bufs=3, side="right")
            s2r = s2r_cm.__enter__()

            qkT_cm = tc.tile_pool(name="qkTp", bufs=1, side="right")
            qkTp = qkT_cm.__enter__()
            qT = qkTp.tile([128, 8, SQ], BF16, tag="qT")
            kT = qkTp.tile([128, 8, S], BF16, tag="kT")

            def qk_rope(wA, boff, tts, dest, out_eng, vengs):
                """QKV DoubleRow matmul + shift-bias row + rope + transpose."""
                for tt in tts:
                    veng = vengs[tt % len(vengs)]
                    ct = s2c.tile([128, 2, 8, 32], BF16, tag="cosr", name="ct")
                    nc.gpsimd.dma_start(
                        out=ct,
                        in_=cosr[tt * 128 : (tt + 1) * 128, :].rearrange(
                            "p (a b c) -> p a b c", b=8, c=32
                        ),
                    )
                    st = s2c.tile([128, 2, 8, 32], BF16, tag="sinr", name="st")
                    nc.gpsimd.dma_start(
                        out=st,
                        in_=sinr[tt * 128 : (tt + 1) * 128, :].rearrange(
                            "p (a b c) -> p a b c", b=8, c=32
                        ),
                    )
                    pt = s2ps.tile([128, 2, 512], FP32, tag="qkvp", name="qkvp")
                    for fc in range(2):
                        for j in range(4):
                            nc.tensor.matmul(
                                pt[:, fc, :],
                                xmT[:, 2 * j : 2 * j + 2,
                                    tt * 128 : (tt + 1) * 128],
                                wA[:, 2 * j : 2 * j + 2,
                                   fc * 512 : (fc + 1) * 512],
                                start=(j == 0),
                                stop=False,
                                perf_mode=PM.DoubleRow,
                            )
                        nc.tensor.matmul(
                            pt[:, fc, :],
                            ones8,
                            bq_sb[:, boff + fc * 512 : boff + (fc + 1) * 512],
                            start=False,
                            stop=True,
                        )
                    qsb = s2r.tile([128, 2, 8, HD], BF16, tag="qsb", name="qsb")
                    nc.scalar.copy(
                        qsb, pt.rearrange("p f (a b) -> p f a b", b=HD)
                    )
                    ro = s2r.tile([128, 2, 8, HD], BF16, tag="rope", name="ro")
                    ta = s2r.tile([128, 2, 8, 32], BF16, tag="ta", name="ta")
                    tb = s2r.tile([128, 2, 8, 32], BF16, tag="tb", name="tb")
                    veng.tensor_mul(ta, qsb[:, :, :, 0:32], ct)
                    veng.tensor_mul(tb, qsb[:, :, :, 32:64], st)
                    veng.tensor_sub(ro[:, :, :, 0:32], ta, tb)
                    ta2 = s2r.tile([128, 2, 8, 32], BF16, tag="ta2", name="ta2")
                    tb2 = s2r.tile([128, 2, 8, 32], BF16, tag="tb2", name="tb2")
                    veng.tensor_mul(ta2, qsb[:, :, :, 32:64], ct)
                    veng.tensor_mul(tb2, qsb[:, :, :, 0:32], st)
                    veng.tensor_add(ro[:, :, :, 32:64], ta2, tb2)
                    rof = ro.rearrange("p a b c -> p (a b c)")
                    ptp = s2tp.tile([128, 8, 128], BF16, tag="tp2", name="ptp")
                    for blk in range(8):
                        nc.tensor.transpose(
                            ptp[:, blk, :],
                            rof[:, blk * 128 : (blk + 1) * 128],
                            ident,
                        )
                    out_eng(
                        dest[:, :, tt * 128 : (tt + 1) * 128],
                        ptp,
                    )

            qk_rope(wAq, 0, range(NTQ), qT, dve_copy, [nc.vector, nc.gpsimd])
            s2wq_cm.__exit__(None, None, None)

            v_cm = tc.tile_pool(name="vp", bufs=1, side="right")
            vp = v_cm.__enter__()
            v_ext = vp.tile([128, 8, H, HD + 1], BF16, tag="vext")
            nc.vector.tensor_copy(
                v_ext[:, :, :, HD : HD + 1],
                ones128[:, 0:128].rearrange("p (a b c) -> p a b c", a=8, b=H),
            )

            # K and V interleaved per token tile so attention (exp) can start
            # as soon as the first K/V tiles land, ~15us earlier.
            s2pv_cm = tc.tile_pool(name="s2psv", bufs=1, space="PSUM")
            s2psv = s2pv_cm.__enter__()
            vengs = [nc.vector, nc.gpsimd]
            for tt in range(NT):
                qk_rope(wAk, D, [tt], kT, act_copy, [vengs[tt % 2]])
                ptv = s2psv.tile([128, 2, 512], FP32, tag="vvp", name="vvp")
                for fc in range(2):
                    for j in range(4):
                        nc.tensor.matmul(
                            ptv[:, fc, :],
                            xmT[:, 2 * j : 2 * j + 2,
                                tt * 128 : (tt + 1) * 128],
                            wB[:, 2 * j : 2 * j + 2,
                               fc * 512 : (fc + 1) * 512],
                            start=(j == 0),
                            stop=False,
                            perf_mode=PM.DoubleRow,
                        )
                    nc.tensor.matmul(
                        ptv[:, fc, :],
                        ones8,
                        bq_sb[:, 2 * D + fc * 512 : 2 * D + (fc + 1) * 512],
                        start=False,
                        stop=True,
                    )
                # v with 1/SW fold; single copy per token tile, on DVE so
                # the ACT stream stays pure-Exp into attention
                nc.vector.tensor_scalar(
                    out=v_ext[:, tt, :, 0:HD],
                    in0=ptv.rearrange("p f (a b) -> p (f a) b", b=HD),
                    scalar1=RSW,
                    scalar2=None,
                    op0=ALU.mult,
                )
            s2wk_cm.__exit__(None, None, None)

            s2pv_cm.__exit__(None, None, None)
            s2tp_cm.__exit__(None, None, None)
            s2ps_cm.__exit__(None, None, None)
            xmT_cm.__exit__(None, None, None)

            # ------------------------------------------------ stage 3: attention
            xm2T_cm = tc.tile_pool(name="xm2Tp", bufs=1)
            xm2Tp = xm2T_cm.__enter__()
            xm2T = xm2Tp.tile([128, 8, SQ], FP8, tag="xm2T")

            attnT_cm = tc.tile_pool(name="attnTp", bufs=1)
            attnTp = attnT_cm.__enter__()
            attnT = attnTp.tile([128, 8, SQ], FP8, tag="attnT")

            # prefetch out-proj + MLP weights during attention
            s4wo_cm = tc.tile_pool(name="s4wo", bufs=1)
            s4wo = s4wo_cm.__enter__()
            wo = s4wo.tile([128, 8, D], FP8, tag="wo")
            nc.sync.dma_start(
                out=wo, in_=wout[:, :].rearrange("(a p) n -> p a n", p=128)
            )
            # chunked so interleaved small DMAs (cos/sin tiles, x rows) are not
            # starved on the serial DMA device by one long transfer
            w1s = s6w1.tile([128, 8, MLP], FP8, tag="w1s")
            for g in range(4):
                nc.sync.dma_start(
                    out=w1s[:, 2 * g : 2 * g + 2, :],
                    in_=w1d[g * 256 : (g + 1) * 256, :].rearrange(
                        "(a p) m -> p a m", p=128
                    ),
                )
            w2s = s6w2.tile([128, 32, D], FP8, tag="w2s")
            for g in range(4):
                nc.sync.dma_start(
                    out=w2s[:, g * 8 : (g + 1) * 8, :],
                    in_=w2d[g * 1024 : (g + 1) * 1024, :].rearrange(
                        "(a p) n -> p a n", p=128
                    ),
                )

            nc.gpsimd.dma_start(out=b1g_sb, in_=b1g[:, :])

            # residual x tiles prefetch on the (idle) pool queue
            s4xs_cm = tc.tile_pool(name="s4xs", bufs=1)
            s4xs = s4xs_cm.__enter__()
            xs4 = s4xs.tile([128, NTQ, D], BF16, tag="xs4")
            for i in range(NTQ):
                nc.gpsimd.dma_start(
                    out=xs4[:, i, :], in_=xb[i * 128 : (i + 1) * 128, :]
                )

            # scores carry SW^2 from fp8 q/k weights; fold into the exp scale
            exp_scale = 0.125 / (SW * SW)

            with (
                tc.tile_pool(name="s3st", bufs=2, space="PSUM") as s3st,
                tc.tile_pool(name="s3pv", bufs=2, space="PSUM") as s3pv,
                tc.tile_pool(name="s3bc", bufs=2, space="PSUM") as s3bc,
                tc.tile_pool(name="s3pr", bufs=4) as s3pr,
                tc.tile_pool(name="s3re", bufs=2) as s3re,
            ):
                for h in range(H):
                    r0 = (h % 2) * 64
                    dc = h // 2
                    pv = s3pv.tile([HD + 1, 512], FP32, tag="pv")
                    for t2 in range(NT // 2):
                        # score pair in one 2-bank PSUM tile -> single exp op
                        stp = s3st.tile([128, 2, 512], FP32, tag="st")
                        for u in range(2):
                            tt = t2 * 2 + u
                            nc.tensor.matmul(
                                stp[:, u, :],
                                kT[r0 : r0 + 64, dc, tt * 128 : (tt + 1) * 128],
                                qT[r0 : r0 + 64, dc, :],
                                start=True,
                                stop=True,
                            )
                        pr = s3pr.tile([128, 2, 512], BF16, tag="pr")
                        nc.scalar.activation(
                            out=pr, in_=stp, func=AF.Exp, scale=exp_scale
                        )
                        for u in range(2):
                            tt = t2 * 2 + u
                            nc.tensor.matmul(
                                pv,
                                v_ext[:, tt, h, :],
                                pr[:, u, :],
                                start=(tt == 0),
                                stop=(tt == NT - 1),
                            )
                    rec = s3re.tile([1, 512], BF16, tag="rec")
                    with nc.allow_low_precision(reason="softmax denom reciprocal"):
                        nc.vector.reciprocal(rec, pv[HD : HD + 1, :])
                    bc = s3bc.tile([64, 512], FP32, tag="bc")
                    nc.tensor.matmul(
                        bc, ones_r[:, 0:64], rec, start=True, stop=True
                    )
                    bcs = s3re.tile([64, 512], BF16, tag="bcs")
                    nc.vector.tensor_copy(bcs, bc)
                    nc.vector.tensor_mul(attnT[r0 : r0 + 64, dc, :], pv[0:HD, :], bcs)

            v_cm.__exit__(None, None, None)
            qkT_cm.__exit__(None, None, None)
            s2r_cm.__exit__(None, None, None)
            s2c_cm.__exit__(None, None, None)

            # ------------------- stage 4+5: out-proj + residual + LN2, per tile
            x1_cm = tc.tile_pool(name="x1p", bufs=1, side="right")
            x1p = x1_cm.__enter__()
            x1 = x1p.tile([128, NTQ, D], FP32, tag="x1")

            with (
                tc.tile_pool(name="s4ps", bufs=4, space="PSUM") as s4ps,
                tc.tile_pool(name="s5st", bufs=4) as s5st,
                tc.tile_pool(name="s5xm", bufs=3) as s5xm,
                tc.tile_pool(name="s5tp", bufs=3, space="PSUM", side="right") as s5tp,
            ):
                def outproj(i):
                    for oc in range(2):
                        pt = s4ps.tile([128, 512], FP32, tag="op")
                        for j in range(4):
                            nc.tensor.matmul(
                                pt,
                                attnT[:, 2 * j : 2 * j + 2, i * 128 : (i + 1) * 128],
                                wo[:, 2 * j : 2 * j + 2, oc * 512 : (oc + 1) * 512],
                                start=(j == 0),
                                stop=(j == 3),
                                perf_mode=PM.DoubleRow,
                            )
                        # x1 = x + (attn @ (wout*gate*SW)) / SW; the two halves
                        # split across engines to unclog the DVE queue here
                        if oc == 0:
                            nc.vector.scalar_tensor_tensor(
                                out=x1[:, i, 0:512],
                                in0=pt,
                                scalar=RSW,
                                in1=xs4[:, i, 0:512],
                                op0=ALU.mult,
                                op1=ALU.add,
                            )
                        else:
                            tmp = s5xm.tile([128, 512], FP32, tag="optmp")
                            nc.scalar.activation(
                                out=tmp, in_=pt, func=AF.Copy, scale=RSW
                            )
                            nc.gpsimd.tensor_add(
                                x1[:, i, 512:1024], tmp, xs4[:, i, 512:1024]
                            )

                def ln2(i):
                    ln_transpose(
                        x1[:, i, :], s5st, s5tp, s5xm,
                        xm2T[:, :, i * 128 : (i + 1) * 128],
                        act_copy if i % 2 == 0 else dve_copy,
                    )

                # software-pipelined: LN2(i-1) emits after outproj(i) so the
                # in-order PE queue isn't blocked by the LN chain
                for i in range(NTQ):
                    outproj(i)
                    if i >= 1:
                        ln2(i - 1)
                ln2(NTQ - 1)

            s4xs_cm.__exit__(None, None, None)
            s4wo_cm.__exit__(None, None, None)
            attnT_cm.__exit__(None, None, None)

            # ------------------------------------------------ stage 6: MLP
            with (
                tc.tile_pool(name="s6h", bufs=1) as s6h,
                tc.tile_pool(name="s6ps", bufs=4, space="PSUM") as s6ps,
                tc.tile_pool(name="s6ps2", bufs=1, space="PSUM") as s6ps2,
                tc.tile_pool(name="s6o", bufs=3) as s6o,
            ):
                hT = s6h.tile([128, 32, SQ], FP8, tag="hT")

                def fc2_psum_tiles():
                    return [
                        s6ps2.tile([128, 512], FP32, tag=f"fc2_{i}", name=f"fc2_{i}")
                        for i in range(NTQ)
                    ]

                def fc2_step(pts, j, oc):
                    for i in range(NTQ):
                        nc.tensor.matmul(
                            pts[i],
                            hT[:, 2 * j : 2 * j + 2, i * 128 : (i + 1) * 128],
                            w2s[:, 2 * j : 2 * j + 2, oc * 512 : (oc + 1) * 512],
                            start=(j == 0),
                            stop=(j == 15),
                            perf_mode=PM.DoubleRow,
                        )

                def fc2_epilogue(pts, oc):
                    for i in range(NTQ):
                        ot = s6o.tile([128, 512], FP32, tag="outs", name="outs")
                        # out = x1 + (h @ (w2*gate*SW)) / SW, fused on DVE
                        nc.vector.scalar_tensor_tensor(
                            out=ot,
                            in0=pts[i],
                            scalar=RSW,
                            in1=x1[:, i, oc * 512 : (oc + 1) * 512],
                            op0=ALU.mult,
                            op1=ALU.add,
                        )
                        nc.sync.dma_start(
                            out=outd[i * 128 : (i + 1) * 128, oc * 512 : (oc + 1) * 512],
                            in_=ot,
                        )

                pts0 = fc2_psum_tiles()
                for mg in range(8):
                    for mi in range(4):
                        mc = mg * 4 + mi
                        pt = s6ps.tile([128, 512], FP32, tag="fc1", name="fc1")
                        # token-halved so the first half's matmuls can start
                        # while LN2 of the last token tiles is still running
                        for half in range(2):
                            for j in range(4):
                                nc.tensor.matmul(
                                    pt[:, half * 256 : (half + 1) * 256],
                                    w1s[:, 2 * j : 2 * j + 2,
                                        mc * 128 : (mc + 1) * 128],
                                    xm2T[:, 2 * j : 2 * j + 2,
                                         half * 256 : (half + 1) * 256],
                                    start=(j == 0),
                                    stop=(j == 3),
                                    perf_mode=PM.DoubleRow,
                                )
                        # gelu((xm2 @ (w1*m2*SW))/SW + shift2 @ w1), exact bias
                        nc.scalar.activation(
                            out=hT[:, mc, :],
                            in_=pt,
                            func=AF.Gelu_apprx_tanh,
                            scale=RSW,
                            bias=b1g_sb[:, mc : mc + 1],
                        )
                        # fc2 for oc=0 chases fc1 chunk-by-chunk (pairs)
                        if mc % 2 == 1:
                            fc2_step(pts0, mc // 2, 0)
                fc2_epilogue(pts0, 0)

                # oc=1: i-outer so each token tile's epilogue + store starts
                # as soon as its accumulation chain completes
                pts1 = fc2_psum_tiles()
                for i in range(NTQ):
                    for j in range(16):
                        nc.tensor.matmul(
                            pts1[i],
                            hT[:, 2 * j : 2 * j + 2, i * 128 : (i + 1) * 128],
                            w2s[:, 2 * j : 2 * j + 2, 512:1024],
                            start=(j == 0),
                            stop=(j == 15),
                            perf_mode=PM.DoubleRow,
                        )
                    ot = s6o.tile([128, 512], FP32, tag="outs", name="outs")
                    nc.vector.scalar_tensor_tensor(
                        out=ot, in0=pts1[i], scalar=RSW,
                        in1=x1[:, i, 512:1024], op0=ALU.mult, op1=ALU.add,
                    )
                    nc.sync.dma_start(
                        out=outd[i * 128 : (i + 1) * 128, 512:1024], in_=ot
                    )

            xm2T_cm.__exit__(None, None, None)
            x1_cm.__exit__(None, None, None)
            s6w2_cm.__exit__(None, None, None)
            s6w1_cm.__exit__(None, None, None)
            const_cm.__exit__(None, None, None)

        for _rep in range(reps):
            _emit_body()

    return nc


_NC_CACHE = {}


def _get_nc(reps=1):
    if reps not in _NC_CACHE:
        _NC_CACHE[reps] = _build_nc(reps)
    return _NC_CACHE[reps]


def _make_in_maps(x, c, norm1_w, norm2_w, w_qkv, w_out, w1, b1, w2, b2,
                  adaLN_w, adaLN_b, cos, sin):
    f32 = lambda a: np.ascontiguousarray(np.asarray(a), dtype=np.float32)
    bf16 = lambda a: np.ascontiguousarray(
        np.asarray(a, dtype=np.float32).astype(ml_dtypes.bfloat16)
    )
    fp8 = lambda a: np.ascontiguousarray(
        np.asarray(a, dtype=np.float32).astype(ml_dtypes.float8_e4m3)
    )
    x = np.asarray(x, dtype=np.float32)
    c = np.asarray(c, dtype=np.float32)
    w_qkv = f32(w_qkv); w_out = f32(w_out); w1 = f32(w1); w2 = f32(w2)
    cos_rep = np.tile(f32(cos), (1, 16))  # [S, 512]
    sin_rep = np.tile(f32(sin), (1, 16))

    # adaLN modulation computed host-side in fp32 and folded into the weights
    mod = c @ f32(adaLN_w) + f32(adaLN_b)  # [B, 6D]
    sm, scm, gm, s2m, sc2, g2 = np.split(mod, 6, axis=-1)

    in_maps = []
    per_batch = {}
    for core in range(N_CORES):
        b, half = core // 2, core % 2
        sh = -half * SQ
        if b not in per_batch:
            m1 = (1.0 + scm[b]) * f32(norm1_w)  # [D]
            m2 = (1.0 + sc2[b]) * f32(norm2_w)
            per_batch[b] = {
                "wqkv": fp8(w_qkv * (m1[:, None] * SW)),
                "bqkv": fp8((sm[b] @ w_qkv)[None, :] * SW),
                "wout": fp8(w_out * (gm[b][None, :] * SW)),
                "w1": fp8(w1 * (m2[:, None] * SW)),
                "b1g": np.ascontiguousarray(
                    ((s2m[b] @ w1) + f32(b1)).reshape(32, 128).T
                ),
                "w2": fp8(w2 * (g2[b][None, :] * SW)),
            }
        in_maps.append(
            dict(
                per_batch[b],
                xb=bf16(np.roll(x[b], sh, axis=0)),
                cosr=bf16(np.roll(cos_rep, sh, axis=0)),
                sinr=bf16(np.roll(sin_rep, sh, axis=0)),
            )
        )
    return in_maps


def _gather(results, x_shape):
    B = x_shape[0]
    out = np.empty(x_shape, dtype=np.float32)
    for core in range(N_CORES):
        b, half = core // 2, core % 2
        out[b, half * SQ : (half + 1) * SQ] = results[core]["out"]
    return out


def run(inputs, trace=False, reps=1):
    nc = _get_nc(reps)
    in_maps = _make_in_maps(**inputs)
    res = run_bass_kernel_spmd(nc, in_maps, list(range(N_CORES)), trace=trace)
    out = _gather(res.results, np.asarray(inputs["x"]).shape)
    return out, res


def kernel(**inputs):
    out, _ = run(inputs)
    return out


# revision 63
# speedup vs baseline: 1.0156x; 1.0156x over previous
"""DiT block kernel for 8 Trainium2 NeuronCores.

Sharding: data-parallel over (batch, seq-half) -> 8 shards, no collectives.
Each core gets x[b] rotated so its 512 query tokens are rows 0:511; K/V are
computed over the full (rotated) 1024-token sequence, so attention needs no
cross-core communication.

v4: the adaLN modulation is folded into the weights HOST-side per core
(c @ adaLN_w is a tiny fp32 matmul): scale rows of wqkv/w1, shift becomes a
bias row (extra rank-1 matmul for QKV, an exact fp32 ACT bias for fc1), and
the msa/mlp gates scale the columns of wout/w2. On-device: LayerNorm via
bn_stats, fp8e4 DoubleRow GEMMs (4x fewer PE cycles), bf16 transposes / rope
/ attention. Weights pre-scaled x64 into e4m3's normal range; 1/64 folded
into downstream scales (exp scale, gelu scale, residual-add scale).
"""

import sys

sys.path.insert(0, "/opt/trn_rl_repo")

import ml_dtypes
import numpy as np

import concourse.bass as bass
import concourse.mybir as mybir
from concourse.bass_utils import run_bass_kernel_spmd
from concourse.masks import make_identity
from concourse.tile import TileContext
from concourse.vector_clock import ScopedClock, VectorClock

# ---------------------------------------------------------------------------
# Walrus in this container caps sync-wait commands per CTRL instruction at a
# small number; Tile's stock tail drain collects one wait per live proc and
# trips that cap. Split the final waits across one SP NOP per proc instead.
_orig_drain_and_barrier = TileContext._drain_and_barrier


def _split_drain_and_barrier(self, tick_clock, wait_clock):
    gc_list = list(tick_clock.global_clock)
    for p, tick in enumerate(gc_list):
        if tick > 0:
            partial = [0] * len(gc_list)
            partial[p] = tick
            nop = self.nc.sync.nop()
            wait_clock.add_sem_waits(nop.ins, ScopedClock({None: VectorClock(partial)}))
    drain_inst = self.nc.sync.drain()
    req = ScopedClock({None: tick_clock.global_clock.copy()})
    cur = ScopedClock({None: tick_clock.global_clock.copy()})
    wait_clock.add_sem_waits(drain_inst.ins, req, cur)
    self.nc.all_engine_barrier()
    popped = self.nc._tile_sem_poison_stack.pop()
    assert popped is self._sem_poison
    self.nc.clear_and_free_semaphores(list(self.sems.allocated().values()))
    self.nc.all_engine_barrier()


TileContext._drain_and_barrier = _split_drain_and_barrier

# This walrus also caps waits per *compute/DMA* instruction (the S3_LW struct
# allows a single wait). Intercept every instruction Tile commits to a basic
# block and spill all but the last wait onto preceding same-engine NOPs.
_nop_proto = None


def _get_nop_proto():
    global _nop_proto
    if _nop_proto is None:
        scratch = bass.Bass()
        _nop_proto = scratch.sync.nop().ins
        _nop_proto.sync_info = None
    return _nop_proto


_orig_add_instruction = TileContext._add_instruction


def _add_instruction_capped(self, inst):
    si = inst.sync_info
    if si is not None and si.on_wait is not None and len(si.on_wait) > 1:
        waits = list(si.on_wait)
        si.on_wait = waits[-1:]
        import copy as _copy

        for w in waits[:-1]:
            nop = _copy.deepcopy(_get_nop_proto())
            nop.name = self.nc.get_next_instruction_name()
            nop.engine = inst.engine
            nop.sync_info = mybir.SyncInfo(on_wait=[w], on_update=[])
            _orig_add_instruction(self, nop)
    _orig_add_instruction(self, inst)


TileContext._add_instruction = _add_instruction_capped

# Capture the Tile scheduler's modeled makespan (cost-model ns) per block —
# the only timing signal available in this container (no NTFF profiling).
MODELED_NS = []
from concourse.bass_interp import CoreSim as _CoreSim

_orig_coresim_simulate = _CoreSim.simulate


def _simulate_capture(self, *a, **k):
    r = _orig_coresim_simulate(self, *a, **k)
    try:
        if self.is_scheduling_pass():
            MODELED_NS.append((getattr(self, "name", "?"), int(self.time)))
    except Exception:
        pass
    return r


_CoreSim.simulate = _simulate_capture


def _publish_perfetto_local(self):
    """Dump the scheduling-sim perfetto trace to a local file (no upload)."""
    import os as _os

    if self.perfetto is None:
        return
    path = _os.environ.get("BASS_MODEL_TRACE_PATH", "/tmp/model_trace.pftrace")
    with open(path, "wb") as f:
        f.write(bytes(self.perfetto.take_serialized()))
    print(f"[kernel] modeled trace written to {path}")


_CoreSim.publish_perfetto = _publish_perfetto_local

# ---------------------------------------------------------------------------

FP32 = mybir.dt.float32
BF16 = mybir.dt.bfloat16
FP8 = mybir.dt.float8e4
AF = mybir.ActivationFunctionType
ALU = mybir.AluOpType
AX = mybir.AxisListType
PM = mybir.MatmulPerfMode

D = 1024
H = 16
HD = 64
S = 1024
SQ = 512  # query tokens per core
MLP = 4096
COND = 128
NT = S // 128  # 8 token tiles
NTQ = SQ // 128  # 4 query token tiles
LN_EPS = 1e-5
N_CORES = 8
SW = 64.0  # fp8 weight pre-scale (host side); 1/SW folded into act scales
RSW = 1.0 / SW


def _build_nc(reps=1):
    import os as _os

    trace_sim = bool(_os.environ.get("BASS_MODEL_TRACE"))
    nc = bass.Bass()

    xb = nc.dram_tensor("xb", [S, D], BF16, kind="ExternalInput")
    wqkv = nc.dram_tensor("wqkv", [D, 3 * D], FP8, kind="ExternalInput")
    bqkv = nc.dram_tensor("bqkv", [1, 3 * D], FP8, kind="ExternalInput")
    bvrep = nc.dram_tensor("bvrep", [128, H, HD], BF16, kind="ExternalInput")
    wout = nc.dram_tensor("wout", [D, D], FP8, kind="ExternalInput")
    w1d = nc.dram_tensor("w1", [D, MLP], FP8, kind="ExternalInput")
    b1g = nc.dram_tensor("b1g", [128, 32], FP32, kind="ExternalInput")
    w2d = nc.dram_tensor("w2", [MLP, D], FP8, kind="ExternalInput")
    cosr = nc.dram_tensor("cosr", [S, 512], BF16, kind="ExternalInput")
    sinr = nc.dram_tensor("sinr", [S, 512], BF16, kind="ExternalInput")
    outd = nc.dram_tensor("out", [SQ, D], FP32, kind="ExternalOutput")

    with TileContext(nc, trace_sim=trace_sim) as tc:
        def _emit_body():
            # ------------------------------------------------------ persistent
            const_cm = tc.tile_pool(name="const", bufs=1)
            const = const_cm.__enter__()
            ident = const.tile([128, 128], BF16, tag="ident")
            make_identity(nc, ident)
            # PE p-state warmup: dummy transposes while DMAs stream, so the
            # 3us ramp completes before the first real matmul.
            warm_cm = tc.tile_pool(name="warmps", bufs=1, space="PSUM")
            warmp = warm_cm.__enter__()
            warm = warmp.tile([128, 8, 128], BF16, tag="warm")
            for blk in range(8):
                nc.tensor.transpose(warm[:, blk, :], ident, ident)
            ones128 = const.tile([128, 128], BF16, tag="ones128")
            nc.vector.memset(ones128, 1.0)
            ones_r = const.tile([1, 128], BF16, tag="ones")
            nc.vector.tensor_copy(ones_r, ones128[0:1, :])
            ones8 = const.tile([1, 128], FP8, tag="ones8")
            nc.vector.memset(ones8, 1.0)
            epst = const.tile([128, 1], FP32, tag="eps")
            nc.vector.memset(epst, LN_EPS)
            bq_sb = const.tile([1, 3 * D], FP8, tag="bq")
            nc.gpsimd.dma_start(out=bq_sb, in_=bqkv[:, :])
            bv_sb = const.tile([128, H, HD], BF16, tag="bv")
            nc.gpsimd.dma_start(out=bv_sb, in_=bvrep[:, :, :])
            b1g_sb = const.tile([128, 32], FP32, tag="b1g")

            xmT_cm = tc.tile_pool(name="xmTp", bufs=1)
            xmTp = xmT_cm.__enter__()
            xmT = xmTp.tile([128, 8, S], FP8, tag="xmT")

            s1x_cm = tc.tile_pool(name="s1x", bufs=8, side="right")
            s1x = s1x_cm.__enter__()

            # stage-2 weight pools
            s2wk_cm = tc.tile_pool(name="s2wk", bufs=1)
            s2wk = s2wk_cm.__enter__()
            s2wq_cm = tc.tile_pool(name="s2wq", bufs=1)
            s2wq = s2wq_cm.__enter__()

            xts = []
            xt = s1x.tile([128, D], BF16, tag="xt", name="xt")
            nc.sync.dma_start(out=xt[:, 0:512], in_=xb[0:128, 0:512])
            nc.sync.dma_start(out=xt[:, 512:1024], in_=xb[0:128, 512:1024])
            xts.append(xt)
            for tt in range(1, 4):
                xt = s1x.tile([128, D], BF16, tag="xt", name="xt")
                nc.sync.dma_start(out=xt, in_=xb[tt * 128 : (tt + 1) * 128, :])
                xts.append(xt)

            # q-weight stream interleaves with the remaining x tiles
            wAq = s2wq.tile([128, 8, 1024], FP8, tag="wA", name="wAq")
            nc.sync.dma_start(
                out=wAq, in_=wqkv[:, 0:1024].rearrange("(a p) n -> p a n", p=128)
            )
            for tt in range(4, NT):
                xt = s1x.tile([128, D], BF16, tag="xt", name="xt")
                nc.sync.dma_start(out=xt, in_=xb[tt * 128 : (tt + 1) * 128, :])
                xts.append(xt)

            # k/v weights stream next on the sync queue
            wAk = s2wk.tile([128, 8, 1024], FP8, tag="wAk", name="wAk")
            nc.sync.dma_start(
                out=wAk, in_=wqkv[:, D : 2 * D].rearrange("(a p) n -> p a n", p=128)
            )
            wB = s2wk.tile([128, 8, 1024], FP8, tag="wB", name="wB")
            nc.sync.dma_start(
                out=wB,
                in_=wqkv[:, 2 * D : 3 * D].rearrange("(a p) n -> p a n", p=128),
            )

            # ---------------------------------------------- LN + transpose
            def ln_transpose(xt, stats, tpp, xmp, dest, deng):
                """LayerNorm(xt) (modulation folded into weights host-side),
                transposed into dest (fp8) with one plain copy."""
                bst = stats.tile([128, 2, 6], FP32, tag="bst")
                nc.vector.bn_stats(bst[:, 0, :], xt[:, 0:512])
                nc.vector.bn_stats(bst[:, 1, :], xt[:, 512:1024])
                mv = stats.tile([128, 2], FP32, tag="mv")
                nc.vector.bn_aggr(mv, bst.rearrange("p a b -> p (a b)"))
                std = stats.tile([128, 1], FP32, tag="std")
                nc.scalar.activation(
                    out=std, in_=mv[:, 1:2], func=AF.Sqrt, bias=epst
                )
                rstd = stats.tile([128, 1], FP32, tag="rstd")
                nc.vector.reciprocal(rstd, std)
                bmu = stats.tile([128, 1], FP32, tag="bmu")
                nc.vector.tensor_scalar(
                    out=bmu, in0=mv[:, 0:1], scalar1=rstd, scalar2=-1.0,
                    op0=ALU.mult, op1=ALU.mult,
                )
                xm = xmp.tile([128, D], BF16, tag="xm")
                nc.scalar.activation(
                    out=xm, in_=xt, func=AF.Identity, scale=rstd, bias=bmu
                )
                pt = tpp.tile([128, 8, 128], BF16, tag="tp")
                for blk in range(8):
                    nc.tensor.transpose(
                        pt[:, blk, :], xm[:, blk * 128 : (blk + 1) * 128], ident
                    )
                deng(dest, pt)

            def act_copy(dst, src):
                nc.scalar.copy(dst, src)

            def dve_copy(dst, src):
                nc.vector.tensor_copy(dst, src)

            with (
                tc.tile_pool(name="s1st", bufs=4) as s1st,
                tc.tile_pool(name="s1xm", bufs=3) as s1xm,
                tc.tile_pool(name="s1tp", bufs=2, space="PSUM", side="right") as s1tp,
            ):
                for tt in range(NT):
                    ln_transpose(
                        xts[tt], s1st, s1tp, s1xm,
                        xmT[:, :, tt * 128 : (tt + 1) * 128],
                        act_copy if tt % 2 == 0 else dve_copy,
                    )
            s1x_cm.__exit__(None, None, None)
            warm_cm.__exit__(None, None, None)

            # MLP weight pools reserve early (below the attention-phase pools
            # in the right stack); their DMAs are emitted at attention start.
            s6w1_cm = tc.tile_pool(name="s6w1", bufs=1, side="right")
            s6w1 = s6w1_cm.__enter__()
            s6w2_cm = tc.tile_pool(name="s6w2", bufs=1, side="right")
            s6w2 = s6w2_cm.__enter__()

            s2ps_cm = tc.tile_pool(name="s2ps", bufs=2, space="PSUM")
            s2ps = s2ps_cm.__enter__()
            s2tp_cm = tc.tile_pool(name="s2tp", bufs=2, space="PSUM")
            s2tp = s2tp_cm.__enter__()

            s2c_cm = tc.tile_pool(name="s2c", bufs=3, side="right")
            s2c = s2c_cm.__enter__()
            s2r_cm = tc.tile_pool(name="s2r", bufs=3, side="right")
            s2r = s2r_cm.__enter__()

            qkT_cm = tc.tile_pool(name="qkTp", bufs=1, side="right")
            qkTp = qkT_cm.__enter__()
            qT = qkTp.tile([128, 8, SQ], BF16, tag="qT")
            kT = qkTp.tile([128, 8, S], BF16, tag="kT")

            def qk_rope(wA, boff, tts, dest, out_eng, vengs):
                """QKV DoubleRow matmul + shift-bias row + rope + transpose."""
                for tt in tts:
                    veng = vengs[tt % len(vengs)]
                    ct = s2c.tile([128, 2, 8, 32], BF16, tag="cosr", name="ct")
                    nc.gpsimd.dma_start(
                        out=ct,
                        in_=cosr[tt * 128 : (tt + 1) * 128, :].rearrange(
                            "p (a b c) -> p a b c", b=8, c=32
                        ),
                    )
                    st = s2c.tile([128, 2, 8, 32], BF16, tag="sinr", name="st")
                    nc.gpsimd.dma_start(
                        out=st,
                        in_=sinr[tt * 128 : (tt + 1) * 128, :].rearrange(
                            "p (a b c) -> p a b c", b=8, c=32
                        ),
                    )
                    pt = s2ps.tile([128, 2, 512], FP32, tag="qkvp", name="qkvp")
                    for fc in range(2):
                        for j in range(4):
                            nc.tensor.matmul(
                                pt[:, fc, :],
                                xmT[:, 2 * j : 2 * j + 2,
                                    tt * 128 : (tt + 1) * 128],
                                wA[:, 2 * j : 2 * j + 2,
                                   fc * 512 : (fc + 1) * 512],
                                start=(j == 0),
                                stop=False,
                                perf_mode=PM.DoubleRow,
                            )
                        nc.tensor.matmul(
                            pt[:, fc, :],
                            ones8,
                            bq_sb[:, boff + fc * 512 : boff + (fc + 1) * 512],
                            start=False,
                            stop=True,
                        )
                    qsb = s2r.tile([128, 2, 8, HD], BF16, tag="qsb", name="qsb")
                    nc.scalar.copy(
                        qsb, pt.rearrange("p f (a b) -> p f a b", b=HD)
                    )
                    ro = s2r.tile([128, 2, 8, HD], BF16, tag="rope", name="ro")
                    ta = s2r.tile([128, 2, 8, 32], BF16, tag="ta", name="ta")
                    tb = s2r.tile([128, 2, 8, 32], BF16, tag="tb", name="tb")
                    veng.tensor_mul(ta, qsb[:, :, :, 0:32], ct)
                    veng.tensor_mul(tb, qsb[:, :, :, 32:64], st)
                    veng.tensor_sub(ro[:, :, :, 0:32], ta, tb)
                    ta2 = s2r.tile([128, 2, 8, 32], BF16, tag="ta2", name="ta2")
                    tb2 = s2r.tile([128, 2, 8, 32], BF16, tag="tb2", name="tb2")
                    veng.tensor_mul(ta2, qsb[:, :, :, 32:64], ct)
                    veng.tensor_mul(tb2, qsb[:, :, :, 0:32], st)
                    veng.tensor_add(ro[:, :, :, 32:64], ta2, tb2)
                    rof = ro.rearrange("p a b c -> p (a b c)")
                    ptp = s2tp.tile([128, 8, 128], BF16, tag="tp2", name="ptp")
                    for blk in range(8):
                        nc.tensor.transpose(
                            ptp[:, blk, :],
                            rof[:, blk * 128 : (blk + 1) * 128],
                            ident,
                        )
                    out_eng(
                        dest[:, :, tt * 128 : (tt + 1) * 128],
                        ptp,
                    )

            qk_rope(wAq, 0, range(NTQ), qT, dve_copy, [nc.vector, nc.gpsimd])
            s2wq_cm.__exit__(None, None, None)

            v_cm = tc.tile_pool(name="vp", bufs=1, side="right")
            vp = v_cm.__enter__()
            v_ext = vp.tile([128, 8, H, HD + 1], BF16, tag="vext")
            nc.vector.tensor_copy(
                v_ext[:, :, :, HD : HD + 1],
                ones128[:, 0:128].rearrange("p (a b c) -> p a b c", a=8, b=H),
            )

            # K and V interleaved per token tile so attention (exp) can start
            # as soon as the first K/V tiles land, ~15us earlier.
            s2pv_cm = tc.tile_pool(name="s2psv", bufs=1, space="PSUM")
            s2psv = s2pv_cm.__enter__()
            vengs = [nc.vector, nc.gpsimd]
            for tt in range(NT):
                qk_rope(wAk, D, [tt], kT, act_copy, [vengs[tt % 2]])
                ptv = s2psv.tile([128, 2, 512], FP32, tag="vvp", name="vvp")
                for fc in range(2):
                    for j in range(4):
                        nc.tensor.matmul(
                            ptv[:, fc, :],
                            xmT[:, 2 * j : 2 * j + 2,
                                tt * 128 : (tt + 1) * 128],
                            wB[:, 2 * j : 2 * j + 2,
                               fc * 512 : (fc + 1) * 512],
                            start=(j == 0),
                            stop=(j == 3),
                            perf_mode=PM.DoubleRow,
                        )
                # v = psum/SW + shift-bias (host table), fused on DVE; keeps
                # the ACT stream pure-Exp into attention and saves the PE
                # bias matmuls
                nc.vector.scalar_tensor_tensor(
                    out=v_ext[:, tt, :, 0:HD],
                    in0=ptv.rearrange("p f (a b) -> p (f a) b", b=HD),
                    scalar=RSW,
                    in1=bv_sb,
                    op0=ALU.mult,
                    op1=ALU.add,
                )
            s2wk_cm.__exit__(None, None, None)

            s2pv_cm.__exit__(None, None, None)
            s2tp_cm.__exit__(None, None, None)
            s2ps_cm.__exit__(None, None, None)
            xmT_cm.__exit__(None, None, None)

            # ------------------------------------------------ stage 3: attention
            xm2T_cm = tc.tile_pool(name="xm2Tp", bufs=1)
            xm2Tp = xm2T_cm.__enter__()
            xm2T = xm2Tp.tile([128, 8, SQ], FP8, tag="xm2T")

            attnT_cm = tc.tile_pool(name="attnTp", bufs=1)
            attnTp = attnT_cm.__enter__()
            attnT = attnTp.tile([128, 8, SQ], FP8, tag="attnT")

            # prefetch out-proj + MLP weights during attention
            s4wo_cm = tc.tile_pool(name="s4wo", bufs=1)
            s4wo = s4wo_cm.__enter__()
            wo = s4wo.tile([128, 8, D], FP8, tag="wo")
            nc.sync.dma_start(
                out=wo, in_=wout[:, :].rearrange("(a p) n -> p a n", p=128)
            )
            # chunked so interleaved small DMAs (cos/sin tiles, x rows) are not
            # starved on the serial DMA device by one long transfer
            w1s = s6w1.tile([128, 8, MLP], FP8, tag="w1s")
            for g in range(4):
                nc.sync.dma_start(
                    out=w1s[:, 2 * g : 2 * g + 2, :],
                    in_=w1d[g * 256 : (g + 1) * 256, :].rearrange(
                        "(a p) m -> p a m", p=128
                    ),
                )
            w2s = s6w2.tile([128, 32, D], FP8, tag="w2s")
            for g in range(4):
                nc.sync.dma_start(
                    out=w2s[:, g * 8 : (g + 1) * 8, :],
                    in_=w2d[g * 1024 : (g + 1) * 1024, :].rearrange(
                        "(a p) n -> p a n", p=128
                    ),
                )

            nc.gpsimd.dma_start(out=b1g_sb, in_=b1g[:, :])

            # residual x tiles prefetch on the (idle) pool queue
            s4xs_cm = tc.tile_pool(name="s4xs", bufs=1)
            s4xs = s4xs_cm.__enter__()
            xs4 = s4xs.tile([128, NTQ, D], BF16, tag="xs4")
            for i in range(NTQ):
                nc.gpsimd.dma_start(
                    out=xs4[:, i, :], in_=xb[i * 128 : (i + 1) * 128, :]
                )

            # scores carry SW^2 from fp8 q/k weights; fold into the exp scale
            exp_scale = 0.125 / (SW * SW)

            with (
                tc.tile_pool(name="s3st", bufs=2, space="PSUM") as s3st,
                tc.tile_pool(name="s3pv", bufs=2, space="PSUM") as s3pv,
                tc.tile_pool(name="s3bc", bufs=2, space="PSUM") as s3bc,
                tc.tile_pool(name="s3pr", bufs=4) as s3pr,
                tc.tile_pool(name="s3re", bufs=2) as s3re,
            ):
                for h in range(H):
                    r0 = (h % 2) * 64
                    dc = h // 2
                    pv = s3pv.tile([HD + 1, 512], FP32, tag="pv")
                    for t2 in range(NT // 2):
                        # score pair in one 2-bank PSUM tile -> single exp op
                        stp = s3st.tile([128, 2, 512], FP32, tag="st")
                        for u in range(2):
                            tt = t2 * 2 + u
                            nc.tensor.matmul(
                                stp[:, u, :],
                                kT[r0 : r0 + 64, dc, tt * 128 : (tt + 1) * 128],
                                qT[r0 : r0 + 64, dc, :],
                                start=True,
                                stop=True,
                            )
                        pr = s3pr.tile([128, 2, 512], BF16, tag="pr")
                        nc.scalar.activation(
                            out=pr, in_=stp, func=AF.Exp, scale=exp_scale
                        )
                        for u in range(2):
                            tt = t2 * 2 + u
                            nc.tensor.matmul(
                                pv,
                                v_ext[:, tt, h, :],
                                pr[:, u, :],
                                start=(tt == 0),
                                stop=(tt == NT - 1),
                            )
                    rec = s3re.tile([1, 512], BF16, tag="rec")
                    with nc.allow_low_precision(reason="softmax denom reciprocal"):
                        nc.vector.reciprocal(rec, pv[HD : HD + 1, :])
                    bc = s3bc.tile([64, 512], FP32, tag="bc")
                    nc.tensor.matmul(
                        bc, ones_r[:, 0:64], rec, start=True, stop=True
                    )
                    bcs = s3re.tile([64, 512], BF16, tag="bcs")
                    nc.vector.tensor_copy(bcs, bc)
                    nc.vector.tensor_mul(attnT[r0 : r0 + 64, dc, :], pv[0:HD, :], bcs)

            v_cm.__exit__(None, None, None)
            qkT_cm.__exit__(None, None, None)
            s2r_cm.__exit__(None, None, None)
            s2c_cm.__exit__(None, None, None)

            # ------------------- stage 4+5: out-proj + residual + LN2, per tile
            x1_cm = tc.tile_pool(name="x1p", bufs=1, side="right")
            x1p = x1_cm.__enter__()
            x1 = x1p.tile([128, NTQ, D], FP32, tag="x1")

            with (
                tc.tile_pool(name="s4ps", bufs=4, space="PSUM") as s4ps,
                tc.tile_pool(name="s5st", bufs=4) as s5st,
                tc.tile_pool(name="s5xm", bufs=3) as s5xm,
                tc.tile_pool(name="s5tp", bufs=3, space="PSUM", side="right") as s5tp,
            ):
                def outproj(i):
                    for oc in range(2):
                        pt = s4ps.tile([128, 512], FP32, tag="op")
                        for j in range(4):
                            nc.tensor.matmul(
                                pt,
                                attnT[:, 2 * j : 2 * j + 2, i * 128 : (i + 1) * 128],
                                wo[:, 2 * j : 2 * j + 2, oc * 512 : (oc + 1) * 512],
                                start=(j == 0),
                                stop=(j == 3),
                                perf_mode=PM.DoubleRow,
                            )
                        # x1 = x + (attn @ (wout*gate*SW)) / SW; the two halves
                        # split across engines to unclog the DVE queue here
                        if oc == 0:
                            nc.vector.scalar_tensor_tensor(
                                out=x1[:, i, 0:512],
                                in0=pt,
                                scalar=RSW,
                                in1=xs4[:, i, 0:512],
                                op0=ALU.mult,
                                op1=ALU.add,
                            )
                        else:
                            tmp = s5xm.tile([128, 512], FP32, tag="optmp")
                            nc.scalar.activation(
                                out=tmp, in_=pt, func=AF.Copy, scale=RSW
                            )
                            nc.gpsimd.tensor_add(
                                x1[:, i, 512:1024], tmp, xs4[:, i, 512:1024]
                            )

                def ln2(i):
                    ln_transpose(
                        x1[:, i, :], s5st, s5tp, s5xm,
                        xm2T[:, :, i * 128 : (i + 1) * 128],
                        act_copy if i % 2 == 0 else dve_copy,
                    )

                # software-pipelined: LN2(i-1) emits after outproj(i) so the
                # in-order PE queue isn't blocked by the LN chain
                for i in range(NTQ):
                    outproj(i)
                    if i >= 1:
                        ln2(i - 1)
                ln2(NTQ - 1)

            s4xs_cm.__exit__(None, None, None)
            s4wo_cm.__exit__(None, None, None)
            attnT_cm.__exit__(None, None, None)

            # ------------------------------------------------ stage 6: MLP
            with (
                tc.tile_pool(name="s6h", bufs=1) as s6h,
                tc.tile_pool(name="s6ps", bufs=4, space="PSUM") as s6ps,
                tc.tile_pool(name="s6ps2", bufs=1, space="PSUM") as s6ps2,
                tc.tile_pool(name="s6o", bufs=3) as s6o,
            ):
                hT = s6h.tile([128, 32, SQ], FP8, tag="hT")

                def fc2_psum_tiles():
                    return [
                        s6ps2.tile([128, 512], FP32, tag=f"fc2_{i}", name=f"fc2_{i}")
                        for i in range(NTQ)
                    ]

                def fc2_step(pts, j, oc):
                    for i in range(NTQ):
                        nc.tensor.matmul(
                            pts[i],
                            hT[:, 2 * j : 2 * j + 2, i * 128 : (i + 1) * 128],
                            w2s[:, 2 * j : 2 * j + 2, oc * 512 : (oc + 1) * 512],
                            start=(j == 0),
                            stop=(j == 15),
                            perf_mode=PM.DoubleRow,
                        )

                def fc2_epilogue(pts, oc):
                    for i in range(NTQ):
                        ot = s6o.tile([128, 512], FP32, tag="outs", name="outs")
                        # out = x1 + (h @ (w2*gate*SW)) / SW, fused on DVE
                        nc.vector.scalar_tensor_tensor(
                            out=ot,
                            in0=pts[i],
                            scalar=RSW,
                            in1=x1[:, i, oc * 512 : (oc + 1) * 512],
                            op0=ALU.mult,
                            op1=ALU.add,
                        )
                        nc.sync.dma_start(
                            out=outd[i * 128 : (i + 1) * 128, oc * 512 : (oc + 1) * 512],
                            in_=ot,
                        )

                pts0 = fc2_psum_tiles()
                for mg in range(8):
                    for mi in range(4):
                        mc = mg * 4 + mi
                        pt = s6ps.tile([128, 512], FP32, tag="fc1", name="fc1")
                        # token-halved so the first half's matmuls can start
                        # while LN2 of the last token tiles is still running
                        for half in range(2):
                            for j in range(4):
                                nc.tensor.matmul(
                                    pt[:, half * 256 : (half + 1) * 256],
                                    w1s[:, 2 * j : 2 * j + 2,
                                        mc * 128 : (mc + 1) * 128],
                                    xm2T[:, 2 * j : 2 * j + 2,
                                         half * 256 : (half + 1) * 256],
                                    start=(j == 0),
                                    stop=(j == 3),
                                    perf_mode=PM.DoubleRow,
                                )
                        # gelu((xm2 @ (w1*m2*SW))/SW + shift2 @ w1), exact bias
                        nc.scalar.activation(
                            out=hT[:, mc, :],
                            in_=pt,
                            func=AF.Gelu_apprx_tanh,
                            scale=RSW,
                            bias=b1g_sb[:, mc : mc + 1],
                        )
                        # fc2 for oc=0 chases fc1 chunk-by-chunk (pairs)
                        if mc % 2 == 1:
                            fc2_step(pts0, mc // 2, 0)
                fc2_epilogue(pts0, 0)

                # oc=1: i-outer so each token tile's epilogue + store starts
                # as soon as its accumulation chain completes
                pts1 = fc2_psum_tiles()
                for i in range(NTQ):
                    for j in range(16):
                        nc.tensor.matmul(
                            pts1[i],
                            hT[:, 2 * j : 2 * j + 2, i * 128 : (i + 1) * 128],
                            w2s[:, 2 * j : 2 * j + 2, 512:1024],
                            start=(j == 0),
                            stop=(j == 15),
                            perf_mode=PM.DoubleRow,
                        )
                    ot = s6o.tile([128, 512], FP32, tag="outs", name="outs")
                    nc.vector.scalar_tensor_tensor(
                        out=ot, in0=pts1[i], scalar=RSW,
                        in1=x1[:, i, 512:1024], op0=ALU.mult, op1=ALU.add,
                    )
                    nc.sync.dma_start(
                        out=outd[i * 128 : (i + 1) * 128, 512:1024], in_=ot
                    )

            xm2T_cm.__exit__(None, None, None)
            x1_cm.__exit__(None, None, None)
            s6w2_cm.__exit__(None, None, None)
            s6w1_cm.__exit__(None, None, None)
            const_cm.__exit__(None, None, None)

        for _rep in range(reps):
            _emit_body()

    return nc


_NC_CACHE = {}


def _get_nc(reps=1):
    if reps not in _NC_CACHE:
        _NC_CACHE[reps] = _build_nc(reps)
    return _NC_CACHE[reps]


def _make_in_maps(x, c, norm1_w, norm2_w, w_qkv, w_out, w1, b1, w2, b2,
                  adaLN_w, adaLN_b, cos, sin):
    f32 = lambda a: np.ascontiguousarray(np.asarray(a), dtype=np.float32)
    bf16 = lambda a: np.ascontiguousarray(
        np.asarray(a, dtype=np.float32).astype(ml_dtypes.bfloat16)
    )
    fp8 = lambda a: np.ascontiguousarray(
        np.asarray(a, dtype=np.float32).astype(ml_dtypes.float8_e4m3)
    )
    x = np.asarray(x, dtype=np.float32)
    c = np.asarray(c, dtype=np.float32)
    w_qkv = f32(w_qkv); w_out = f32(w_out); w1 = f32(w1); w2 = f32(w2)
    cos_rep = np.tile(f32(cos), (1, 16))  # [S, 512]
    sin_rep = np.tile(f32(sin), (1, 16))

    # adaLN modulation computed host-side in fp32 and folded into the weights
    mod = c @ f32(adaLN_w) + f32(adaLN_b)  # [B, 6D]
    sm, scm, gm, s2m, sc2, g2 = np.split(mod, 6, axis=-1)

    in_maps = []
    per_batch = {}
    for core in range(N_CORES):
        b, half = core // 2, core % 2
        sh = -half * SQ
        if b not in per_batch:
            m1 = (1.0 + scm[b]) * f32(norm1_w)  # [D]
            m2 = (1.0 + sc2[b]) * f32(norm2_w)
            per_batch[b] = {
                "wqkv": fp8(w_qkv * (m1[:, None] * SW)),
                "bqkv": fp8((sm[b] @ w_qkv)[None, :] * SW),
                "bvrep": bf16(
                    np.tile((sm[b] @ w_qkv[:, 2 * 1024 :])[None, :], (128, 1))
                ).reshape(128, 16, 64),
                "wout": fp8(w_out * (gm[b][None, :] * SW)),
                "w1": fp8(w1 * (m2[:, None] * SW)),
                "b1g": np.ascontiguousarray(
                    ((s2m[b] @ w1) + f32(b1)).reshape(32, 128).T
                ),
                "w2": fp8(w2 * (g2[b][None, :] * SW)),
            }
        in_maps.append(
            dict(
                per_batch[b],
                xb=bf16(np.roll(x[b], sh, axis=0)),
                cosr=bf16(np.roll(cos_rep, sh, axis=0)),
                sinr=bf16(np.roll(sin_rep, sh, axis=0)),
            )
        )
    return in_maps


def _gather(results, x_shape):
    B = x_shape[0]
    out = np.empty(x_shape, dtype=np.float32)
    for core in range(N_CORES):
        b, half = core // 2, core % 2
        out[b, half * SQ : (half + 1) * SQ] = results[core]["out"]
    return out


def run(inputs, trace=False, reps=1):
    nc = _get_nc(reps)
    in_maps = _make_in_maps(**inputs)
    res = run_bass_kernel_spmd(nc, in_maps, list(range(N_CORES)), trace=trace)
    out = _gather(res.results, np.asarray(inputs["x"]).shape)
    return out, res


def kernel(**inputs):
    out, _ = run(inputs)
    return out
